# revision 25
# baseline (speedup 1.0000x reference)
"""Deep-hedging GRU kernel for 8 Trainium2 NeuronCores.

Data-parallel over n_sim: 16384 paths -> 2048 per core.  Feature-major
layout: h is [H=128 partitions, 2048 paths free]; 63 steps fully unrolled.

v2: float32r gin matmuls (1 cyc/row), bf16 hh/W_out matmuls + bf16 hidden
state, pair-wide (1024-col) DVE blend ops, 3 consolidated DMAs per step.
"""

import numpy as np
import ml_dtypes

import concourse.bass as bass
import concourse.tile as tile
from concourse import bacc, mybir
from concourse.bass_utils import run_bass_kernel_spmd

F32 = mybir.dt.float32
F32R = mybir.dt.float32r
F16 = mybir.dt.float16
BF16 = mybir.dt.bfloat16
AF = mybir.ActivationFunctionType
OP = mybir.AluOpType

N_CORES = 8
NSIM, NSTEP, IND = 16384, 64, 16
H, O = 128, 8
T = NSTEP - 1            # 63 recurrence steps
P = NSIM // N_CORES      # 2048 paths per core
NT = 4                   # path tiles per core
TN = P // NT             # 512 paths per tile
CAP = 1.0
GATE_DT = 'f32'   # 'f32' | 'f16' | 'bf16'

_cached = {}
_last_results = None


def _build_program():
    nc = bacc.Bacc("TRN2", target_bir_lowering=False, debug=False)

    xp = nc.dram_tensor("xp", [T, 24, NT * TN], F32, kind="ExternalInput")
    wgin = nc.dram_tensor("wgin", [128, 3 * H], F32, kind="ExternalInput")
    whht = nc.dram_tensor("whht", [H, 3 * H], F32, kind="ExternalInput")
    woutt = nc.dram_tensor("woutt", [H, 32], F32, kind="ExternalInput")
    bhn = nc.dram_tensor("bhn", [H, 1], F32, kind="ExternalInput")
    boutp = nc.dram_tensor("boutp", [128, 1], F32, kind="ExternalInput")
    y = nc.dram_tensor("y", [8, NT, O, 8 * TN], F32, kind="ExternalOutput")

    with tile.TileContext(nc) as tc:
        from contextlib import ExitStack

        with ExitStack() as ctx:
            persist = ctx.enter_context(tc.tile_pool(name="persist", bufs=1))
            rzin_pool = ctx.enter_context(
                tc.tile_pool(name="rzin", bufs=2, space="PSUM")
            )
            small_ps = ctx.enter_context(
                tc.tile_pool(name="smallps", bufs=2, space="PSUM")
            )
            sb = ctx.enter_context(tc.tile_pool(name="work", bufs=3))

            w_gin = persist.tile([128, 3 * H], F32, tag="w_gin")
            HDT = F32
            w_hht = persist.tile([H, 3 * H], F32, tag="w_hht")
            w_outt = persist.tile([H, 32], F32, tag="w_outt")
            b_hn = persist.tile([H, 1], F32, tag="b_hn")
            b_outp = persist.tile([128, 1], F32, tag="b_outp")
            nc.sync.dma_start(w_gin[:], wgin.ap())
            nc.sync.dma_start(w_hht[:], whht.ap())
            nc.sync.dma_start(w_outt[:], woutt.ap())
            nc.sync.dma_start(b_hn[:], bhn.ap())
            nc.sync.dma_start(b_outp[:], boutp.ap())

            h_buf = [persist.tile([H, P], F32, tag=f"h{i}", name=f"h{i}") for i in range(2)]
            gin_buf = [persist.tile([32, NT * TN], F32, tag=f"gin{i}", name=f"gin{i}") for i in range(2)]
            pos_buf = [persist.tile([128, TN], F32, tag=f"pos{i}", name=f"pos{i}") for i in range(2)]

            nc.gpsimd.memset(h_buf[0][:], 0.0)
            nc.gpsimd.memset(pos_buf[0][:], 0.0)
            nc.vector.memset(gin_buf[0][0:8, :], 0.0)
            nc.gpsimd.dma_start(gin_buf[0][8:32, :], xp.ap()[0])

            for t in range(T):
                gc = gin_buf[t % 2]
                gn = gin_buf[(t + 1) % 2]
                hc = h_buf[t % 2]
                hnx = h_buf[(t + 1) % 2]
                pc = pos_buf[t % 2]
                pn = pos_buf[(t + 1) % 2]

                if t + 1 < T:
                    nc.gpsimd.dma_start(gn[8:32, :], xp.ap()[t + 1])

                d_ps = small_ps.tile([128, TN], F32, tag="small", name="d_ps")
                for pair in range(2):
                    GDT = {'f32': F32, 'f16': F16, 'bf16': BF16}[GATE_DT]
                    rzp = sb.tile([128, 4 * TN], GDT, tag="rzp", name="rzp")
                    n_pair = sb.tile([128, 2 * TN], GDT, tag="np", name="n_pair")
                    for q in range(2):
                        j = 2 * pair + q
                        cols = slice(TN * j, TN * (j + 1))
                        gsl = gc[0:25, cols]
                        rzin = rzin_pool.tile([128, 3 * TN], F32, tag="rzin", name="rzin")
                        hn_ps = small_ps.tile([128, TN], F32, tag="small", name="hn_ps")

                        hr = hc[:, cols]
                        whv = w_hht[:]
                        nc.tensor.matmul(
                            rzin[:, 0:TN], whv[:, 0:H], hr,
                            start=True, stop=False,
                        )
                        nc.tensor.matmul(
                            rzin[:, TN : 2 * TN], whv[:, H : 2 * H], hr,
                            start=True, stop=False,
                        )
                        nc.tensor.matmul(
                            hn_ps[:], whv[:, 2 * H : 3 * H], hr,
                            start=True, stop=True,
                        )
                        nc.tensor.matmul(
                            rzin[:, 0:TN],
                            w_gin[0:25, 0:H], gsl,
                            start=False, stop=True,
                        )
                        nc.tensor.matmul(
                            rzin[:, TN : 2 * TN],
                            w_gin[0:25, H : 2 * H], gsl,
                            start=False, stop=True,
                        )
                        nc.tensor.matmul(
                            rzin[:, 2 * TN : 3 * TN],
                            w_gin[0:25, 2 * H : 3 * H], gsl,
                            start=True, stop=True,
                        )

                        nc.scalar.activation(
                            rzp[:, 2 * TN * q : 2 * TN * (q + 1)],
                            rzin[:, 0 : 2 * TN], AF.Sigmoid,
                        )
                        t1 = sb.tile([128, TN], GDT, tag="t1", name="t1")
                        nc.vector.scalar_tensor_tensor(
                            t1[:], hn_ps[:], b_hn[:],
                            rzp[:, 2 * TN * q : 2 * TN * q + TN],
                            op0=OP.add, op1=OP.mult,
                        )
                        t2 = sb.tile([128, TN], GDT, tag="t2", name="t2")
                        nc.vector.tensor_add(t2[:], t1[:], rzin[:, 2 * TN : 3 * TN])
                        nc.scalar.activation(
                            n_pair[:, TN * q : TN * (q + 1)], t2[:], AF.Tanh
                        )

                    # pair-wide blend: h' = n + z*(h-n)
                    pcols = slice(2 * TN * pair, 2 * TN * (pair + 1))
                    zv = rzp[:].rearrange("p (a b) -> p a b", a=4)[:, 1::2, :]
                    t3 = sb.tile([128, 2 * TN], GDT, tag="t3", name="t3")
                    nc.vector.tensor_sub(t3[:], hc[:, pcols], n_pair[:])
                    t4 = sb.tile([128, 2 * TN], GDT, tag="t4", name="t4")
                    nc.vector.tensor_tensor(t4[:], zv, t3[:], op=OP.mult)
                    nc.vector.tensor_add(hnx[:, pcols], n_pair[:], t4[:])

                    for q in range(2):
                        j = 2 * pair + q
                        cols = slice(TN * j, TN * (j + 1))
                        wov = w_outt[:]
                        hxv = hnx[:, cols]
                        nc.tensor.matmul(
                            d_ps[32 * j : 32 * (j + 1), :], wov, hxv,
                            start=True, stop=True, tile_position=(0, 32 * j),
                        )

                qv = sb.tile([128, TN], F32, tag="q", name="qv")
                nc.vector.scalar_tensor_tensor(
                    qv[:], d_ps[:], b_outp[:], pc[:], op0=OP.add, op1=OP.add
                )
                nc.vector.tensor_scalar(
                    pn[:], qv[:], -CAP, CAP, op0=OP.max, op1=OP.min
                )
                if t % 8 == 0:
                    dout = persist.tile([128, 8 * TN], F32, tag=f"dout{(t//8)%2}",
                                        name=f"dout{t//8}")
                nc.vector.tensor_sub(
                    dout[:, TN * (t % 8) : TN * (t % 8 + 1)], pn[:], pc[:]
                )
                if t % 8 == 7 or t == T - 1:
                    wcols = TN * (t % 8 + 1)
                    for j in range(NT):
                        nc.gpsimd.dma_start(
                            y.ap()[t // 8, j][:, 0:wcols],
                            dout[32 * j : 32 * j + O, 0:wcols],
                        )
                if t + 1 < T:
                    for j in range(NT):
                        nc.sync.dma_start(
                            gn[0:8, TN * j : TN * (j + 1)],
                            pn[32 * j : 32 * j + 8, :],
                        )
    nc.compile()
    return nc


def _prep_core_inputs(X, W_ih, W_hh, b_ih, b_hh, W_out, b_out):
    X = np.asarray(X, np.float32)
    W_ih = np.asarray(W_ih, np.float32)
    W_hh = np.asarray(W_hh, np.float32)
    b_ih = np.asarray(b_ih, np.float32)
    b_hh = np.asarray(b_hh, np.float32)
    W_out = np.asarray(W_out, np.float32)
    b_out = np.asarray(b_out, np.float32)

    base = np.zeros((32, 3 * H), np.float32)
    base[0:8] = W_ih[:, IND : IND + O].T
    base[8:24] = W_ih[:, 0:IND].T
    bias = np.concatenate(
        [b_ih[0:H] + b_hh[0:H], b_ih[H : 2 * H] + b_hh[H : 2 * H], b_ih[2 * H :]]
    )
    base[24] = bias
    wgin = np.ascontiguousarray(np.tile(base, (NT, 1)))

    whht = np.ascontiguousarray(W_hh.T)
    woutt = np.zeros((H, 32), np.float32)
    woutt[:, :O] = W_out.T
    bhn = np.ascontiguousarray(b_hh[2 * H :].reshape(H, 1))
    brow = np.zeros(32, np.float32)
    brow[:O] = b_out
    boutp = np.ascontiguousarray(np.tile(brow, NT).reshape(128, 1))

    in_maps = []
    for c in range(N_CORES):
        Xc = X[c * P : (c + 1) * P, :T, :]
        xpc = np.zeros((T, 24, NT * TN), np.float32)
        xpc[:, :IND, :] = Xc.transpose(1, 2, 0)
        xpc[:, IND, :] = 1.0
        in_maps.append(
            {
                "xp": xpc,
                "wgin": wgin,
                "whht": whht,
                "woutt": woutt,
                "bhn": bhn,
                "boutp": boutp,
            }
        )
    return in_maps


def kernel(X, W_ih, W_hh, b_ih, b_hh, W_out, b_out):
    global _last_results
    if "nc" not in _cached:
        _cached["nc"] = _build_program()
    nc = _cached["nc"]

    in_maps = _prep_core_inputs(X, W_ih, W_hh, b_ih, b_hh, W_out, b_out)
    res = run_bass_kernel_spmd(nc, in_maps, core_ids=list(range(N_CORES)))
    _last_results = res

    out = np.empty((NSIM, T, O), np.float32)
    for c in range(N_CORES):
        yc = res.results[c]["y"].reshape(8, NT, O, 8, TN)   # [chunk, j, o, t%8, p]
        for ch in range(8):
            for s in range(8):
                t = 8 * ch + s
                if t >= T:
                    break
                blk = yc[ch, :, :, s, :].transpose(0, 2, 1).reshape(P, O)
                out[c * P : (c + 1) * P, t, :] = blk
    return out
